# revision 8
# baseline (speedup 1.0000x reference)
"""Trainium2 Bass kernel for nn_Attention_32839319945876 (sparse_attention).

Head-parallel: 48 heads -> 6 per core on 8 NeuronCores. Host does layout
prep (transposes, wedge folding, RoPE tables); device does projections,
wedged+roped QK^T with causal mask, exp-softmax stats, top-12 threshold via
chunked max8, masked-exp weight matrix, marker matmul, MLP, and per-core
partial output projection combined by ReduceScatter; host reassembles.
"""

import math
import sys
import types

import numpy as np

try:
    import antenv.axon_hooks  # noqa: F401
except Exception:
    _m = types.ModuleType("antenv.axon_hooks")
    _m.get_axon_ntff_profile_hook = lambda: None
    sys.modules["antenv.axon_hooks"] = _m

import concourse.bass as bass
import concourse.tile as tile
from concourse import mybir
from concourse.bass_utils import run_bass_kernel_spmd

B, T, C = 1, 1024, 768
N_HEAD = 12
N_BR = 4
DH = C // N_HEAD          # 64
H_TOT = N_BR * N_HEAD     # 48
K_RET = 12
EPS = 1.1920929e-07
N_CORES = 8
HPC = H_TOT // N_CORES    # 6
NTB = T // 128            # 8
SCALE = DH ** -0.5
SIG_C = math.pi / math.sqrt(3.0)
NEG = -40.0

F32 = mybir.dt.float32
F32R = mybir.dt.float32r
BF16 = mybir.dt.bfloat16
ACTF = mybir.ActivationFunctionType
ALU = mybir.AluOpType

_DUPCOL = np.concatenate([np.arange(0, DH, 2), np.arange(1, DH, 2),
                          np.arange(0, DH, 2), np.arange(1, DH, 2)])


def _rope_tables():
    inv = 1.0 / (10000.0 ** (np.arange(0, DH, 2, dtype=np.float64) / DH))
    ang = np.arange(T, dtype=np.float64)[:, None] * inv[None, :]
    cos, sin = np.cos(ang), np.sin(ang)
    F = np.concatenate([cos.T, -sin.T, sin.T, cos.T], axis=0)
    return F.astype(np.float32)


def _pair_add():
    P = np.zeros((128, DH), np.float32)
    for m in range(32):
        P[m, m] = 1.0
        P[m + 32, m] = 1.0
    for m in range(32, 64):
        P[m + 32, m] = 1.0
        P[m + 64, m] = 1.0
    return P


def _host_prep(A, X, WK_w, WK_b, WQ_w, WQ_b, wedge_A, wedge_bias, sink,
               v_nulls, fc_w, fc_b, proj_w, proj_b, WO, WO_b):
    A = np.asarray(A); X = np.asarray(X)
    WK_w = np.asarray(WK_w); WK_b = np.asarray(WK_b)
    WQ_w = np.asarray(WQ_w); WQ_b = np.asarray(WQ_b)
    wedge_A = np.asarray(wedge_A); wedge_bias = np.asarray(wedge_bias)
    sink = np.asarray(sink); v_nulls = np.asarray(v_nulls)
    fc_w = np.asarray(fc_w); fc_b = np.asarray(fc_b)
    proj_w = np.asarray(proj_w); proj_b = np.asarray(proj_b)
    WO = np.asarray(WO); WO_b = np.asarray(WO_b)

    Askew = wedge_A - wedge_A.T
    F_rope = _rope_tables()
    Padd = _pair_add()
    tri = np.where(np.tril(np.ones((128, 128), bool)), 0.0, NEG).astype(np.float32)
    ident = np.eye(128, dtype=np.float32)
    vre = v_nulls.reshape(H_TOT, DH)
    esink_all = np.exp(sink.astype(np.float64)).astype(np.float32)

    in_maps = []
    for core in range(N_CORES):
        heads = list(range(core * HPC, (core + 1) * HPC))
        Ms = [np.eye(DH, dtype=np.float32) + Askew
              + np.diag(wedge_bias[h]) for h in heads]
        WQ6 = np.concatenate([WQ_w[:, h * DH:(h + 1) * DH] for h in heads], 1)
        WQb6 = np.concatenate([WQ_b[h * DH:(h + 1) * DH] for h in heads])
        bases = [h % N_HEAD for h in heads]
        WK6f = np.concatenate(
            [WK_w[:, b * DH:(b + 1) * DH] @ M for b, M in zip(bases, Ms)], 1)
        WKb6f = np.concatenate(
            [WK_b[b * DH:(b + 1) * DH] @ M for b, M in zip(bases, Ms)])
        WK6p = np.concatenate([WK_w[:, b * DH:(b + 1) * DH] for b in bases], 1)
        WKb6p = np.concatenate([WK_b[b * DH:(b + 1) * DH] for b in bases])
        Mqdup = np.stack([np.concatenate([(M * SCALE)[:, _DUPCOL]] * 2, 0)
                          for M in Ms])  # [6, 128, 128]
        Pdup = np.concatenate([np.eye(DH, dtype=np.float32)[:, _DUPCOL]] * 2, 0)
        vsink = np.stack([vre[h] * esink_all[h] for h in heads])
        WOp = []
        for p in range(HPC // 2):
            h0, h1 = heads[2 * p], heads[2 * p + 1]
            s0 = WO[h0 // N_HEAD][(h0 % N_HEAD) * DH:(h0 % N_HEAD + 1) * DH]
            s1 = WO[h1 // N_HEAD][(h1 % N_HEAD) * DH:(h1 % N_HEAD + 1) * DH]
            WOp.append(np.concatenate([s0, s1], 0) * 0.25)
        WOp = np.stack(WOp)
        yb = (WO_b.mean(0) if core == 0 else np.zeros(C)).astype(np.float32)

        in_maps.append({
            "AT": np.ascontiguousarray(A[0].T).astype(np.float32),
            "XT": np.ascontiguousarray(X[0].T).astype(np.float32),
            "WQ6": WQ6.astype(np.float32),
            "WQb6": WQb6.reshape(1, -1).astype(np.float32),
            "WK6f": WK6f.astype(np.float32),
            "WKb6f": WKb6f.reshape(1, -1).astype(np.float32),
            "WK6p": WK6p.astype(np.float32),
            "WKb6p": WKb6p.reshape(1, -1).astype(np.float32),
            "Mqdup": Mqdup.astype(np.float32),
            "Pdup": Pdup.astype(np.float32),
            "Padd": Padd,
            "Frope": F_rope,
            "tri": tri,
            "ident": ident, "identr": ident,
            "esink": np.tile(np.array([esink_all[h] for h in heads],
                                     np.float32)[None, :], (128, 1)),
            "vsink": vsink.reshape(1, -1).astype(np.float32),
            "fcw": fc_w.astype(np.float32),
            "fcb": fc_b.reshape(1, -1).astype(np.float32),
            "pjw": proj_w.astype(np.float32),
            "pjb": proj_b.reshape(1, -1).astype(np.float32),
            "WOp": WOp.astype(np.float32),
            "yb": yb.reshape(1, -1).astype(np.float32),
        })
    return in_maps


def build_kernel(stage=99):
    from kernel_build import build_kernel as _bk
    return _bk(dict(T=T, C=C, DH=DH, HPC=HPC, NTB=NTB, N_CORES=N_CORES,
                    EPS=EPS, SIG_C=SIG_C), stage=stage)


_NC_CACHE = {}


def kernel(**inputs):
    in_maps = _host_prep(**inputs)
    if "nc" not in _NC_CACHE:
        _NC_CACHE["nc"] = build_kernel()
    nc = _NC_CACHE["nc"]
    res = run_bass_kernel_spmd(nc, in_maps, core_ids=list(range(N_CORES)))
    slabs = [res.results[c]["out"] for c in range(N_CORES)]
    yT = np.concatenate(slabs, axis=0)
    return np.ascontiguousarray(yT.T)[None].astype(np.float32)


# revision 9
# speedup vs baseline: 18.8032x; 18.8032x over previous
"""Trainium2 Bass kernel for nn_Attention_32839319945876 (sparse_attention).

Head-parallel: 48 heads -> 6 per core on 8 NeuronCores. Host does layout
prep (transposes, wedge folding, RoPE tables); device does projections,
wedged+roped QK^T with causal mask, exp-softmax stats, top-12 threshold via
chunked max8, masked-exp weight matrix, marker matmul, MLP, and per-core
partial output projection combined by ReduceScatter; host reassembles.
"""

import math
import sys
import types

import numpy as np

try:
    import antenv.axon_hooks  # noqa: F401
except Exception:
    _m = types.ModuleType("antenv.axon_hooks")
    _m.get_axon_ntff_profile_hook = lambda: None
    sys.modules["antenv.axon_hooks"] = _m

import contextlib

import concourse.bass as bass
import concourse.bacc as bacc
import concourse.tile as tile
from concourse import mybir
from concourse.bass_utils import run_bass_kernel_spmd

B, T, C = 1, 1024, 768
N_HEAD = 12
N_BR = 4
DH = C // N_HEAD          # 64
H_TOT = N_BR * N_HEAD     # 48
K_RET = 12
EPS = 1.1920929e-07
N_CORES = 8
HPC = H_TOT // N_CORES    # 6
NTB = T // 128            # 8
SCALE = DH ** -0.5
SIG_C = math.pi / math.sqrt(3.0)
NEG = -40.0

F32 = mybir.dt.float32
F32R = mybir.dt.float32r
BF16 = mybir.dt.bfloat16
ACTF = mybir.ActivationFunctionType
ALU = mybir.AluOpType

_DUPCOL = np.concatenate([np.arange(0, DH, 2), np.arange(1, DH, 2),
                          np.arange(0, DH, 2), np.arange(1, DH, 2)])


def _rope_tables():
    inv = 1.0 / (10000.0 ** (np.arange(0, DH, 2, dtype=np.float64) / DH))
    ang = np.arange(T, dtype=np.float64)[:, None] * inv[None, :]
    cos, sin = np.cos(ang), np.sin(ang)
    F = np.concatenate([cos.T, -sin.T, sin.T, cos.T], axis=0)
    return F.astype(np.float32)


def _pair_add():
    P = np.zeros((128, DH), np.float32)
    for m in range(32):
        P[m, m] = 1.0
        P[m + 32, m] = 1.0
    for m in range(32, 64):
        P[m + 32, m] = 1.0
        P[m + 64, m] = 1.0
    return P


def _host_prep(A, X, WK_w, WK_b, WQ_w, WQ_b, wedge_A, wedge_bias, sink,
               v_nulls, fc_w, fc_b, proj_w, proj_b, WO, WO_b):
    A = np.asarray(A); X = np.asarray(X)
    WK_w = np.asarray(WK_w); WK_b = np.asarray(WK_b)
    WQ_w = np.asarray(WQ_w); WQ_b = np.asarray(WQ_b)
    wedge_A = np.asarray(wedge_A); wedge_bias = np.asarray(wedge_bias)
    sink = np.asarray(sink); v_nulls = np.asarray(v_nulls)
    fc_w = np.asarray(fc_w); fc_b = np.asarray(fc_b)
    proj_w = np.asarray(proj_w); proj_b = np.asarray(proj_b)
    WO = np.asarray(WO); WO_b = np.asarray(WO_b)

    Askew = wedge_A - wedge_A.T
    F_rope = _rope_tables()
    Padd = _pair_add()
    tri = np.where(np.tril(np.ones((128, 128), bool)), 0.0, NEG).astype(np.float32)
    ident = np.eye(128, dtype=np.float32)
    vre = v_nulls.reshape(H_TOT, DH)
    esink_all = np.exp(sink.astype(np.float64)).astype(np.float32)

    in_maps = []
    for core in range(N_CORES):
        heads = list(range(core * HPC, (core + 1) * HPC))
        Ms = [np.eye(DH, dtype=np.float32) + Askew
              + np.diag(wedge_bias[h]) for h in heads]
        WQ6 = np.concatenate([WQ_w[:, h * DH:(h + 1) * DH] for h in heads], 1)
        WQb6 = np.concatenate([WQ_b[h * DH:(h + 1) * DH] for h in heads])
        bases = [h % N_HEAD for h in heads]
        WK6f = np.concatenate(
            [WK_w[:, b * DH:(b + 1) * DH] @ M for b, M in zip(bases, Ms)], 1)
        WKb6f = np.concatenate(
            [WK_b[b * DH:(b + 1) * DH] @ M for b, M in zip(bases, Ms)])
        WK6p = np.concatenate([WK_w[:, b * DH:(b + 1) * DH] for b in bases], 1)
        WKb6p = np.concatenate([WK_b[b * DH:(b + 1) * DH] for b in bases])
        Mqdup = np.stack([np.concatenate([(M * SCALE)[:, _DUPCOL]] * 2, 0)
                          for M in Ms])  # [6, 128, 128]
        Pdup = np.concatenate([np.eye(DH, dtype=np.float32)[:, _DUPCOL]] * 2, 0)
        vsink = np.stack([vre[h] * esink_all[h] for h in heads])
        WOp = []
        for p in range(HPC // 2):
            h0, h1 = heads[2 * p], heads[2 * p + 1]
            s0 = WO[h0 // N_HEAD][(h0 % N_HEAD) * DH:(h0 % N_HEAD + 1) * DH]
            s1 = WO[h1 // N_HEAD][(h1 % N_HEAD) * DH:(h1 % N_HEAD + 1) * DH]
            WOp.append(np.concatenate([s0, s1], 0) * 0.25)
        WOp = np.stack(WOp)
        yb = (WO_b.mean(0) if core == 0 else np.zeros(C)).astype(np.float32)

        in_maps.append({
            "AT": np.ascontiguousarray(A[0].T).astype(np.float32),
            "XT": np.ascontiguousarray(X[0].T).astype(np.float32),
            "WQ6": WQ6.astype(np.float32),
            "WQb6": WQb6.reshape(1, -1).astype(np.float32),
            "WK6f": WK6f.astype(np.float32),
            "WKb6f": WKb6f.reshape(1, -1).astype(np.float32),
            "WK6p": WK6p.astype(np.float32),
            "WKb6p": WKb6p.reshape(1, -1).astype(np.float32),
            "Mqdup": Mqdup.astype(np.float32),
            "Pdup": Pdup.astype(np.float32),
            "Padd": Padd,
            "Frope": F_rope,
            "tri": tri,
            "ident": ident, "identr": ident,
            "esink": np.tile(np.array([esink_all[h] for h in heads],
                                     np.float32)[None, :], (128, 1)),
            "vsink": vsink.reshape(1, -1).astype(np.float32),
            "fcw": fc_w.astype(np.float32),
            "fcb": fc_b.reshape(1, -1).astype(np.float32),
            "pjw": proj_w.astype(np.float32),
            "pjb": proj_b.reshape(1, -1).astype(np.float32),
            "WOp": WOp.astype(np.float32),
            "yb": yb.reshape(1, -1).astype(np.float32),
        })
    return in_maps


def build_kernel(stage=99):
    return _build_kernel_impl(dict(T=T, C=C, DH=DH, HPC=HPC, NTB=NTB,
                                   N_CORES=N_CORES, EPS=EPS, SIG_C=SIG_C),
                              stage=stage)


def _build_kernel_impl(cfg, stage=99):
    T, C, DH, HPC, NTB, N_CORES = (cfg["T"], cfg["C"], cfg["DH"], cfg["HPC"],
                                   cfg["NTB"], cfg["N_CORES"])
    EPS, SIG_C = cfg["EPS"], cfg["SIG_C"]
    NCH = C // 128
    NPAIR = HPC // 2

    nc = bacc.Bacc(target_bir_lowering=False, debug=False)
    dp = lambda n, s: nc.declare_dram_parameter(n, list(s), F32, isOutput=False)
    dr = lambda n, s: nc.declare_dram_parameter(n, list(s), F32R, isOutput=False)
    AT = dr("AT", (C, T)); XT = dr("XT", (C, T))
    WQ6 = dr("WQ6", (C, HPC * DH)); WQb6 = dp("WQb6", (1, HPC * DH))
    WK6f = dr("WK6f", (C, HPC * DH)); WKb6f = dp("WKb6f", (1, HPC * DH))
    WK6p = dr("WK6p", (C, HPC * DH)); WKb6p = dp("WKb6p", (1, HPC * DH))
    Mqdup = dp("Mqdup", (HPC, 128, 128))   # M''_h stacked twice
    Pdup = dp("Pdup", (128, 128))          # dup matrix stacked twice
    Padd = dp("Padd", (128, DH)); Frope = dp("Frope", (128, T))
    tri = dp("tri", (128, 128)); ident = dp("ident", (128, 128))
    esink = dp("esink", (128, HPC)); vsink = dp("vsink", (1, HPC * DH))
    fcw = dp("fcw", (DH, 4 * DH)); fcb = dp("fcb", (1, 4 * DH))
    pjw = dp("pjw", (4 * DH, DH)); pjb = dp("pjb", (1, DH))
    WOp = dp("WOp", (NPAIR, 128, C)); yb = dp("yb", (1, C))
    out = nc.declare_dram_parameter("out", [C // N_CORES, T], F32,
                                    isOutput=True)
    y_bounce = nc.dram_tensor("y_bounce", [C, T], F32)
    y_rs = nc.dram_tensor("y_rs", [C // N_CORES, T], F32)

    with tile.TileContext(nc) as tc:
        ctx = contextlib.ExitStack()
        with ctx:
            cpool = ctx.enter_context(tc.tile_pool(name="consts", bufs=1))
            wpool = ctx.enter_context(tc.tile_pool(name="weights", bufs=1))
            persist = ctx.enter_context(tc.tile_pool(name="persist", bufs=1))
            work = ctx.enter_context(tc.tile_pool(name="work", bufs=2))
            ework = ctx.enter_context(tc.tile_pool(name="ework", bufs=2))
            tiny = ctx.enter_context(tc.tile_pool(name="tiny", bufs=4))
            ps_junk = ctx.enter_context(
                tc.tile_pool(name="ps_junk", bufs=1, space="PSUM"))

            junk_ps = ps_junk.tile([32, 32], BF16)

            def load_f32(pool, ap, shape, name):
                t = pool.tile(list(shape), F32, name=name)
                nc.sync.dma_start(t[:], ap)
                return t

            ident_f = load_f32(cpool, ident[:, :], (128, 128), "ident_f")
            ident_b = cpool.tile([128, 128], BF16)
            nc.scalar.copy(ident_b[:], ident_f[:])

            def presync_w(psum_ap):
                """Junk PE write into a PSUM slot: absorbs the slot's WAR
                deps into PE program order so the following self-loading
                f32r matmul needs at most one sync wait."""
                nc.tensor.matmul(psum_ap.bitcast(BF16)[0:32, 0:32],
                                 ident_b[0:32, 0:32], ident_b[0:32, 0:32],
                                 is_transpose=True, start=True, stop=True)

            obs_scr = cpool.tile([1, 8], F32)

            def observe(ap, col):
                nc.vector.tensor_copy(obs_scr[0:1, col:col + 1], ap[0:1, 0:1])

            def presync_r(ap):
                nc.tensor.matmul(junk_ps[:],
                                 ap.bitcast(BF16)[0:32, 0:32],
                                 ident_b[0:32, 0:32],
                                 is_transpose=True, start=True, stop=True)

            def loadr(pool, ap, shape, name, tag=None):
                kw = {"tag": tag} if tag else {}
                t = pool.tile(list(shape), F32R, name=name, **kw)
                nc.sync.dma_start(t[:], ap)
                presync_r(t[:])
                return t

            # ---------------- small constants / weights ----------------
            def loadf(pool, ap, shape, name):
                t = load_f32(pool, ap, shape, name)
                presync_r(t[:])
                return t

            mqd_sb = [loadf(wpool, Mqdup[h, :, :], (128, 128),
                            f"mqd{h}") for h in range(HPC)]
            pdup_sb = loadf(wpool, Pdup[:, :], (128, 128), "pdup_r")
            padd_sb = loadf(wpool, Padd[:, :], (128, DH), "padd_r")
            frope_sb = load_f32(cpool, Frope[:, :], (128, T), "frope_sb")
            observe(frope_sb, 0)
            tri_sb = load_f32(cpool, tri[:, :], (128, 128), "tri_sb")
            observe(tri_sb, 1)
            esink_sb = load_f32(cpool, esink[:, :], (128, HPC), "esink_sb")
            observe(esink_sb, 2)

            def to_b(pool, ap, shape, name):
                t = pool.tile(list(shape), BF16, name=name)
                nc.gpsimd.dma_start(t[:], ap)
                return t

            wqb_b = to_b(wpool, WQb6[:, :], (1, HPC * DH), "wqb_b")
            wkbf_b = to_b(wpool, WKb6f[:, :], (1, HPC * DH), "wkbf_b")
            wkbp_b = to_b(wpool, WKb6p[:, :], (1, HPC * DH), "wkbp_b")
            vsink_b = to_b(wpool, vsink[:, :], (1, HPC * DH), "vsink_b")
            fcw_b = to_b(wpool, fcw[:, :], (DH, 4 * DH), "fcw_b")
            fcb_b = to_b(wpool, fcb[:, :], (1, 4 * DH), "fcb_b")
            pjw_b = [to_b(wpool, pjw[u * 128:(u + 1) * 128, :], (128, DH),
                          f"pjw_b{u}") for u in range(2)]
            pjb_b = to_b(wpool, pjb[:, :], (1, DH), "pjb_b")
            wop_b = [to_b(wpool, WOp[p, :, :], (128, C), f"wop_b{p}")
                     for p in range(NPAIR)]
            yb_b = to_b(wpool, yb[:, :], (1, C), "yb_b")
            ones_b = cpool.tile([1, T], BF16)
            nc.vector.memset(ones_b[:], 1.0)

            # ---------------- stage B: projections + transposes --------
            kp_slab = [persist.tile([128, HPC * DH], BF16, name=f"kp{tb}")
                       for tb in range(NTB)]
            qkpool_cm = tc.tile_pool(name="qkpool", bufs=1)
            qkpool = qkpool_cm.__enter__()
            actpool_cm = tc.tile_pool(name="actpool", bufs=1)
            actpool = actpool_cm.__enter__()
            qT = [qkpool.tile([128, T], F32, name=f"qTs{p}")
                  for p in range(NPAIR)]
            kT = [qkpool.tile([128, T], F32, name=f"kTs{p}")
                  for p in range(NPAIR)]
            wq_sb = [loadr(qkpool, WQ6[c * 128:(c + 1) * 128, :],
                           (128, HPC * DH), f"wq{c}", tag=f"wx{c}")
                     for c in range(NCH)]

            with tc.tile_pool(name="ps_b", bufs=2, space="PSUM") as ps_b:
                # ---- q pass ----
                a_sb = [loadr(actpool, AT[c * 128:(c + 1) * 128, :],
                              (128, T), f"at{c}", tag=f"act{c}")
                        for c in range(NCH)]
                for tb in range(NTB):
                    ts_ = slice(tb * 128, (tb + 1) * 128)
                    q_ps = ps_b.tile([128, HPC * DH], F32, tag="proj",
                                     name="q_ps")
                    presync_w(q_ps[:])
                    for c in range(NCH):
                        nc.tensor.matmul(q_ps[:], a_sb[c][:, ts_],
                                         wq_sb[c][:],
                                         start=(c == 0), stop=False)
                    nc.tensor.matmul(q_ps[:], ones_b[:, 0:128], wqb_b[:],
                                     start=False, stop=True)
                    q2 = work.tile([128, HPC * DH], F32, tag="q2", name="q2")
                    nc.scalar.activation(q2[:], q_ps[:], ACTF.Square)
                    ssq = tiny.tile([128, HPC], F32, tag="ssq", name="ssq")
                    nc.vector.reduce_sum(
                        ssq[:], q2[:].rearrange("p (h d) -> p h d", d=DH),
                        axis=mybir.AxisListType.X)
                    nc.vector.tensor_scalar(ssq[:], ssq[:], 1.0 / DH, EPS,
                                            ALU.mult, ALU.add)
                    nc.scalar.activation(ssq[:], ssq[:], ACTF.Sqrt)
                    rin = tiny.tile([128, HPC], F32, tag="rin", name="rin")
                    nc.vector.reciprocal(rin[:], ssq[:])
                    qs = work.tile([128, HPC * DH], F32, tag="qs", name="qs")
                    for h in range(HPC):
                        hsl = slice(h * DH, (h + 1) * DH)
                        nc.vector.tensor_scalar(qs[:, hsl], q_ps[:, hsl],
                                                rin[:, h:h + 1], None,
                                                ALU.mult)
                    for hh in range(HPC):
                        cs = slice(hh * DH, (hh + 1) * DH)
                        rs_ = slice((hh % 2) * DH, (hh % 2) * DH + DH)
                        tp = ps_b.tile([DH, 128], F32, tag="tp", name="tp")
                        presync_w(tp[:])
                        nc.tensor.transpose(tp[:], qs[:, cs], ident_f[:])
                        nc.vector.tensor_copy(qT[hh // 2][rs_, ts_], tp[:])
                # ---- k pass (reuses act slots) ----
                x_sb = [loadr(actpool, XT[c * 128:(c + 1) * 128, :],
                              (128, T), f"xt{c}", tag=f"act{c}")
                        for c in range(NCH)]
                wkf_sb = [loadr(qkpool, WK6f[c * 128:(c + 1) * 128, :],
                                (128, HPC * DH), f"wkf{c}", tag=f"wx{c}")
                          for c in range(NCH)]
                for tb in range(NTB):
                    ts_ = slice(tb * 128, (tb + 1) * 128)
                    kf_ps = ps_b.tile([128, HPC * DH], F32, tag="proj",
                                      name="kf_ps")
                    presync_w(kf_ps[:])
                    for c in range(NCH):
                        nc.tensor.matmul(kf_ps[:], x_sb[c][:, ts_],
                                         wkf_sb[c][:],
                                         start=(c == 0), stop=False)
                    nc.tensor.matmul(kf_ps[:], ones_b[:, 0:128], wkbf_b[:],
                                     start=False, stop=True)
                    kf_sb = work.tile([128, HPC * DH], F32, tag="kf_sb",
                                      name="kf_sb")
                    nc.scalar.copy(kf_sb[:], kf_ps[:])
                    for hh in range(HPC):
                        cs = slice(hh * DH, (hh + 1) * DH)
                        rs_ = slice((hh % 2) * DH, (hh % 2) * DH + DH)
                        tp2 = ps_b.tile([DH, 128], F32, tag="tp", name="tp2")
                        presync_w(tp2[:])
                        nc.tensor.transpose(tp2[:], kf_sb[:, cs], ident_f[:])
                        nc.vector.tensor_copy(kT[hh // 2][rs_, ts_], tp2[:])
                wkp_sb = [loadr(qkpool, WK6p[c * 128:(c + 1) * 128, :],
                                (128, HPC * DH), f"wkp{c}", tag=f"wx{c}")
                          for c in range(NCH)]
                for tb in range(NTB):
                    ts_ = slice(tb * 128, (tb + 1) * 128)
                    kp_ps = ps_b.tile([128, HPC * DH], F32, tag="proj",
                                      name="kp_ps")
                    presync_w(kp_ps[:])
                    for c in range(NCH):
                        nc.tensor.matmul(kp_ps[:], x_sb[c][:, ts_],
                                         wkp_sb[c][:],
                                         start=(c == 0), stop=False)
                    nc.tensor.matmul(kp_ps[:], ones_b[:, 0:128], wkbp_b[:],
                                     start=False, stop=True)
                    nc.scalar.copy(kp_slab[tb][:], kp_ps[:])
            actpool_cm.__exit__(None, None, None)

            # ---------------- stage D: wedge + rope --------------------
            qTr = [persist.tile([128, T], F32R, name=f"qTr{p}")
                   for p in range(NPAIR)]
            kTr = [persist.tile([128, T], F32R, name=f"kTr{p}")
                   for p in range(NPAIR)]
            with tc.tile_pool(name="ps_d", bufs=2, space="PSUM") as ps_d:
                for h in range(HPC if stage >= 2 else 0):
                    pair, half = h // 2, h % 2
                    rs_ = slice(half * DH, half * DH + DH)
                    for (src, lhs, dst) in ((qT, mqd_sb[h], qTr),
                                            (kT, pdup_sb, kTr)):
                        xd = ps_d.tile([128, T], F32, tag="xd", name="xd")
                        presync_w(xd[:])
                        for nh in range(2):
                            ns = slice(nh * 512, (nh + 1) * 512)
                            nc.tensor.matmul(xd[:, ns], lhs[rs_, :],
                                             src[pair][rs_, ns],
                                             start=True, stop=True)
                        xr = work.tile([128, T], F32, tag="xrope",
                                       name="xr")
                        nc.vector.tensor_tensor(xr[:], xd[:], frope_sb[:],
                                                ALU.mult)
                        rr = ps_d.tile([DH, T], F32, tag="rr", bufs=1,
                                       name="rr")
                        presync_w(rr[:])
                        for nh in range(2):
                            ns = slice(nh * 512, (nh + 1) * 512)
                            nc.tensor.matmul(rr[:, ns], padd_sb[:],
                                             xr[:, ns],
                                             start=True, stop=True)
                        nc.vector.tensor_copy(dst[pair][rs_, :], rr[:])
            qkpool_cm.__exit__(None, None, None)

            # ---------------- stage E: per-head attention --------------
            ctx_slab = [persist.tile([128, T], BF16, name=f"ctx{p}")
                        for p in range(NPAIR)]
            with (
                tc.tile_pool(name="ps_e1", bufs=1, space="PSUM") as ps_e1,
                tc.tile_pool(name="ps_e2", bufs=2, space="PSUM") as ps_e2,
                tc.tile_pool(name="ps_e3", bufs=1, space="PSUM") as ps_e3,
            ):
                for h in range(HPC):
                    pair, half = h // 2, h % 2
                    rs_ = slice(half * DH, half * DH + DH)
                    hsl = slice(h * DH, (h + 1) * DH)
                    kpa = [ework.tile([128, DH + 1], BF16, tag=f"kpa{j}",
                                      name=f"kpa{h}_{j}")
                           for j in range(NTB)]
                    for j in range(NTB):
                        nc.scalar.copy(kpa[j][:, 0:DH], kp_slab[j][:, hsl])
                        nc.vector.memset(kpa[j][:, DH:DH + 1], 1.0)
                    for i in range(NTB):
                        L = (i + 1) * 128
                        ts_ = slice(i * 128, (i + 1) * 128)
                        s_ps = ps_e1.tile([128, 1024], F32, tag="s_ps",
                                          name="s_ps")
                        presync_w(s_ps[:])
                        for n0 in range(0, L, 512):
                            n1 = min(n0 + 512, L)
                            nc.tensor.matmul(s_ps[:, n0:n1],
                                             qTr[pair][rs_, ts_],
                                             kTr[pair][rs_, n0:n1],
                                             start=True, stop=True)
                        nc.vector.tensor_tensor(s_ps[:, ts_], s_ps[:, ts_],
                                                tri_sb[:], ALU.add)
                        e_sb = ework.tile([128, 1024], F32, tag="e_sb",
                                          name="e_sb")
                        zrow = tiny.tile([128, 1], F32, tag="zrow",
                                         name="zrow")
                        nc.scalar.activation(e_sb[:, 0:L], s_ps[:, 0:L],
                                             ACTF.Exp, accum_out=zrow[:])
                        m8a = tiny.tile([128, 8], F32, tag="m8a", name="m8a")
                        m8b = tiny.tile([128, 8], F32, tag="m8b", name="m8b")
                        nc.vector.max(m8a[:], e_sb[:, 0:L])
                        r1f = ework.tile([128, 1024], F32, tag="r1f",
                                         name="r1f")
                        nc.vector.match_replace(r1f[:, 0:L], m8a[:],
                                                e_sb[:, 0:L], 0.0)
                        nc.vector.max(m8b[:], r1f[:, 0:L])
                        th_f = m8b[:, 3:4]
                        w_sb = ework.tile([128, 1024], BF16, tag="w_sb",
                                          name="w_sb")
                        msk = ework.tile([128, 1024], BF16, tag="msk",
                                         name="msk")
                        nc.vector.tensor_scalar(msk[:, 0:L], e_sb[:, 0:L],
                                                th_f, None, ALU.is_ge)
                        nc.vector.tensor_tensor(w_sb[:, 0:L], e_sb[:, 0:L],
                                                msk[:, 0:L], ALU.mult)
                        mk_ps = ps_e3.tile([128, DH + 1], F32, tag="mk_ps",
                                           name="mk_ps")
                        for j in range(i + 1):
                            js = slice(j * 128, (j + 1) * 128)
                            wt_ps = ps_e2.tile([128, 128], BF16, tag="sm",
                                               name="wt_ps")
                            nc.tensor.transpose(wt_ps[:], w_sb[:, js],
                                                ident_b[:])
                            wt_sb = ework.tile([128, 128], BF16, tag="wt_sb",
                                               name="wt_sb")
                            nc.scalar.copy(wt_sb[:], wt_ps[:])
                            nc.tensor.matmul(mk_ps[:], wt_sb[:], kpa[j][:],
                                             start=(j == 0), stop=(j == i))
                        zf = tiny.tile([128, 1], F32, tag="zf", name="zf")
                        nc.vector.tensor_scalar(zf[:], zrow[:],
                                                esink_sb[:, h:h + 1],
                                                None, ALU.add)
                        den = tiny.tile([128, 1], F32, tag="den", name="den")
                        nc.vector.scalar_tensor_tensor(
                            den[:], zf[:], 1e-9, mk_ps[:, DH:DH + 1],
                            ALU.mult, ALU.add)
                        nu = tiny.tile([128, 1], F32, tag="nu", name="nu")
                        nc.vector.reciprocal(nu[:], den[:])
                        rz = tiny.tile([128, 1], F32, tag="rz", name="rz")
                        nc.vector.reciprocal(rz[:], zf[:])
                        mkn = tiny.tile([128, DH], BF16, tag="mkn",
                                        name="mkn")
                        nc.vector.tensor_scalar(mkn[:], mk_ps[:, 0:DH],
                                                nu[:], None, ALU.mult)
                        mt_ps = ps_e2.tile([DH, 128], BF16, tag="sm",
                                           name="mt_ps")
                        nc.tensor.transpose(mt_ps[:], mkn[:], ident_b[:])
                        mknT = tiny.tile([DH, 128], BF16, tag="mknT",
                                         name="mknT")
                        nc.scalar.copy(mknT[:], mt_ps[:])
                        h_ps = ps_e3.tile([128, 4 * DH], F32, tag="h_ps",
                                          name="h_ps")
                        nc.tensor.matmul(h_ps[:], mknT[:], fcw_b[:],
                                         start=True, stop=False)
                        nc.tensor.matmul(h_ps[:], ones_b[:, 0:128], fcb_b[:],
                                         start=False, stop=True)
                        t1 = work.tile([128, 4 * DH], BF16, tag="t1",
                                       name="t1")
                        nc.vector.tensor_scalar(t1[:], h_ps[:], 0.75, 1.0,
                                                ALU.mult, ALU.add)
                        hsq = work.tile([128, 4 * DH], BF16, tag="hsq",
                                        name="hsq")
                        nc.scalar.activation(hsq[:], h_ps[:], ACTF.Square)
                        g = work.tile([128, 4 * DH], BF16, tag="g", name="g")
                        nc.vector.tensor_tensor(g[:], hsq[:], t1[:], ALU.mult)
                        gsq = work.tile([128, 4 * DH], BF16, tag="gsq",
                                        name="gsq")
                        ssq2 = tiny.tile([128, 1], F32, tag="ssq2",
                                         name="ssq2")
                        nc.scalar.activation(gsq[:], g[:], ACTF.Square,
                                             accum_out=ssq2[:])
                        nc.vector.tensor_scalar(ssq2[:], ssq2[:],
                                                1.0 / (4 * DH), EPS,
                                                ALU.mult, ALU.add)
                        nc.scalar.activation(ssq2[:], ssq2[:], ACTF.Sqrt)
                        ni = tiny.tile([128, 1], F32, tag="ni", name="ni")
                        nc.vector.reciprocal(ni[:], ssq2[:])
                        nsc = tiny.tile([128, 1], F32, tag="nsc", name="nsc")
                        nc.vector.tensor_scalar(nsc[:], ni[:], SIG_C, None,
                                                ALU.mult)
                        sig = work.tile([128, 4 * DH], BF16, tag="sig",
                                        name="sig")
                        nc.scalar.activation(sig[:], g[:], ACTF.Sigmoid,
                                             scale=nsc[:])
                        u = work.tile([128, 4 * DH], BF16, tag="u", name="u")
                        nc.vector.scalar_tensor_tensor(u[:], g[:], ni[:],
                                                       sig[:], ALU.mult,
                                                       ALU.mult)
                        ot_ps = ps_e3.tile([DH, 128], F32, tag="ot_ps",
                                           name="ot_ps")
                        for ub in range(2):
                            us = slice(ub * 128, (ub + 1) * 128)
                            ut_ps = ps_e2.tile([128, 128], BF16, tag="sm",
                                               name="ut_ps")
                            nc.tensor.transpose(ut_ps[:], u[:, us],
                                                ident_b[:])
                            utsb = work.tile([128, 128], BF16, tag="utsb",
                                             name="utsb")
                            nc.scalar.copy(utsb[:], ut_ps[:])
                            nc.tensor.matmul(ot_ps[:], pjw_b[ub][:], utsb[:],
                                             start=(ub == 0), stop=False)
                        rzb = tiny.tile([128, 1], BF16, tag="rzb", name="rzb")
                        nc.vector.tensor_copy(rzb[:], rz[:])
                        rzt_ps = ps_e2.tile([1, 128], BF16, tag="sm",
                                            name="rzt_ps")
                        nc.tensor.transpose(rzt_ps[:], rzb[:], ident_b[:])
                        rzrow = tiny.tile([1, 128], BF16, tag="rzrow",
                                          name="rzrow")
                        nc.scalar.copy(rzrow[:], rzt_ps[:])
                        nc.tensor.matmul(ot_ps[:], pjb_b[:], ones_b[:, 0:128],
                                         start=False, stop=False)
                        nc.tensor.matmul(ot_ps[:], vsink_b[:, hsl], rzrow[:],
                                         start=False, stop=True)
                        nc.scalar.copy(ctx_slab[pair][rs_, ts_], ot_ps[:])

            # ---------------- stage F: output projection + RS ----------
            with (
                tc.tile_pool(name="ps_f", bufs=2, space="PSUM") as ps_f,
                tc.tile_pool(name="fpool", bufs=2) as fpool,
            ):
                for ob in range(NCH if stage >= 4 else 0):
                    obs = slice(ob * 128, (ob + 1) * 128)
                    y_ps = ps_f.tile([128, T], F32, tag="y_ps", name="y_ps")
                    for p in range(NPAIR):
                        for nh in range(2):
                            ns = slice(nh * 512, (nh + 1) * 512)
                            nc.tensor.matmul(y_ps[:, ns], wop_b[p][:, obs],
                                             ctx_slab[p][:, ns],
                                             start=(p == 0), stop=False)
                    for nh in range(2):
                        ns = slice(nh * 512, (nh + 1) * 512)
                        nc.tensor.matmul(y_ps[:, ns], yb_b[:, obs],
                                         ones_b[:, ns],
                                         start=False, stop=(nh == 1))
                    y_sb = fpool.tile([128, T], F32, tag="y_sb", name="y_sb")
                    nc.scalar.copy(y_sb[:], y_ps[:])
                    nc.sync.dma_start(y_bounce[obs, :], y_sb[:])
                if stage >= 5:
                    nc.gpsimd.collective_compute(
                        "ReduceScatter", ALU.add,
                        ins=[y_bounce.ap().opt()],
                        outs=[y_rs.ap().opt()],
                        replica_groups=[list(range(N_CORES))],
                    )
                    rs_sb = fpool.tile([C // N_CORES, T], F32, tag="y_sb",
                                       name="rs_sb")
                    nc.sync.dma_start(rs_sb[:], y_rs[:, :])
                    nc.sync.dma_start(out[:, :], rs_sb[:])
                else:
                    dbg = fpool.tile([C // N_CORES, T], F32, tag="y_sb",
                                     name="dbg")
                    if stage >= 4:
                        nc.sync.dma_start(dbg[:], y_bounce[0:C // N_CORES, :])
                    elif stage >= 3:
                        nc.vector.tensor_copy(dbg[0:96, :],
                                              ctx_slab[0][0:96, :])
                    elif stage >= 2:
                        nc.vector.tensor_copy(dbg[0:96, :], qTr[0][0:96, :])
                    else:
                        nc.vector.tensor_copy(dbg[0:96, 0:384],
                                              kp_slab[0][0:96, :])
                    nc.sync.dma_start(out[:, :], dbg[:])
    nc.finalize()
    return nc



_NC_CACHE = {}


def kernel(**inputs):
    in_maps = _host_prep(**inputs)
    if "nc" not in _NC_CACHE:
        _NC_CACHE["nc"] = build_kernel()
    nc = _NC_CACHE["nc"]
    res = run_bass_kernel_spmd(nc, in_maps, core_ids=list(range(N_CORES)))
    slabs = [res.results[c]["out"] for c in range(N_CORES)]
    yT = np.concatenate(slabs, axis=0)
    return np.ascontiguousarray(yT.T)[None].astype(np.float32)
